# revision 1
# baseline (speedup 1.0000x reference)
"""Trainium2 Bass kernel for nn_Attention_39676907884025.

out[b, q, :] = (1/SK) * sum_k value[b, k, :] for every q: q_param (1x1) is
broadcast over query and key, the score matrix is constant along the softmax
axis, and softmax of a constant row is exactly uniform. Only `value` touches
the device; batch B=16 is data-parallel over 8 cores (2 per core).

Raw bacc, hand-scheduled. HW-measured 27.3-29.3us vs 27.6-32.2us for the
tile-framework baseline across device thermal states (paired same-state
runs: 4+us faster). Design notes:
  - DMA only on the two HWDGE rings (SWDGE/gpsimd queues measure ~40%
    slower per descriptor and steal SDMA slots): 4 quarter-chunk loads +
    4 quarter stores per batch (256 KB, 2 KB descriptors), one dedicated
    completion semaphore per load chunk (shared counting sems are racy:
    the 16 SDMA engine slots increment independently).
  - DVE tree-reduces chunks as they land, dependent ops kept >= 2 apart
    (interleaved order + 8-col spacer copies) so bacc inserts none of its
    ~450 ns same-engine pipeline drains; final fold casts the accumulator
    to bf16 for free (rel err 1.7e-3 << 2e-2 tolerance).
  - PE: one single-pass bf16 matmul per batch (constant 1/SK stationary)
    reduces across partitions AND broadcasts the mean into fp32 PSUM.
  - ACT alone widens PSUM -> (128, 512) via four independent PSUM reads
    (no dependent pair -> no ACT drain) and is the only engine waiting on
    the PE semaphore: DVE/Pool waiting on PE event-accel semaphores
    deadlocks the device, as does ending Pool with in-flight SWDGE DMAs.


out[b, q, :] = (1/SK) * sum_k value[b, k, :]  for every q (softmax of a
constant score matrix is exactly uniform).

Raw bacc. Per core (2 batches), proven-safe sync patterns only:
  - 4 load chunks per batch (256 KB, 2 KB descriptors) over 3 queues,
    dedicated sem per chunk.
  - DVE: interleaved tree schedule with hazard distance >= 2 between
    dependent ops (plus tiny dummy spacers), so bacc inserts no ~450 ns
    pipeline drains; final fold writes acc in bf16 (output cast is free,
    |mean| error ~0.3% << 2e-2 budget).
  - PE: one 1-pass bf16 matmul per batch (constant 1/SK stationary) ->
    fp32 PSUM mean tile broadcast to all 128 rows.
  - ACT alone widens PSUM -> (128, 512) SBUF (s_mm has a single waiter;
    DVE/PE never wait on event-accel sems from PE - that pattern hangs
    the hardware, see kernels 3-6/9).
  - stores: 4 per batch (256 KB, 2 KB descriptors): sync/act one each,
    gpsimd two.
"""

import sys

import numpy as np

if "/opt/trn_rl_repo" not in sys.path:
    sys.path.insert(0, "/opt/trn_rl_repo")

B, SQ, SK, D, DV = 16, 2048, 2048, 128, 128
N_CORES = 8
BPC = B // N_CORES  # batches per core
P = 128

LAST_RESULT = None  # BassKernelResults of the most recent run (for profiling)


def _build_nc():
    import concourse.bacc as bacc
    import concourse.mybir as mybir

    f32 = mybir.dt.float32
    bf16 = mybir.dt.bfloat16
    nc = bacc.Bacc("TRN2", target_bir_lowering=False)

    val = nc.dram_tensor("value", [BPC, SK, DV], f32, kind="ExternalInput")
    out = nc.dram_tensor("out", [BPC, SQ, DV], f32, kind="ExternalOutput")

    w = nc.alloc_sbuf_tensor("w_const", [P, P], bf16)
    xts = [nc.alloc_sbuf_tensor(f"xt{b}", [P, SK], f32) for b in range(BPC)]
    # level-1 tree scratch per chunk c: [256c, 256c+256)
    lv1 = [nc.alloc_sbuf_tensor(f"lv1_{b}", [P, 1024], f32) for b in range(BPC)]
    # per-chunk 128-col tree sums at [128c, 128c+128)
    qac = [nc.alloc_sbuf_tensor(f"qac_{b}", [P, 512], f32) for b in range(BPC)]
    # fold partials: F1 at [0:128], F2 at [128:256]
    pr = [nc.alloc_sbuf_tensor(f"pr_{b}", [P, 256], f32) for b in range(BPC)]
    acc = [nc.alloc_sbuf_tensor(f"acc_{b}", [P, P], bf16) for b in range(BPC)]
    wide = [nc.alloc_sbuf_tensor(f"wide{b}", [P, 512], f32) for b in range(BPC)]
    dum = nc.alloc_sbuf_tensor("dum", [P, 128], f32)
    pss = [nc.alloc_psum_tensor(f"ps{b}", [P, P], f32) for b in range(BPC)]

    s_ld = [
        [nc.alloc_semaphore(f"s_ld_{b}_{c}") for c in range(4)] for b in range(BPC)
    ]
    s_sp = nc.alloc_semaphore("s_sp")
    s_act = nc.alloc_semaphore("s_act")
    s_w = nc.alloc_semaphore("s_w")
    s_dve = nc.alloc_semaphore("s_dve")
    s_mm = nc.alloc_semaphore("s_mm")
    s_wide = nc.alloc_semaphore("s_wide")

    def xdst(b):
        return xts[b][:].rearrange("p (t d) -> p t d", d=DV)

    def xsrc(b):
        return val[b].rearrange("(p t) d -> p t d", p=P)

    def odst(b):
        return out[b].rearrange("(p t) d -> p t d", p=P)

    def wsrc(b):
        return wide[b][:].rearrange("p (t d) -> p t d", d=DV)

    def load(eng, b, c):
        return eng.dma_start(
            xdst(b)[:, 4 * c : 4 * c + 4, :], xsrc(b)[:, 4 * c : 4 * c + 4, :]
        ).then_inc(s_ld[b][c], 16)

    def store(eng, b, t0, sem):
        return eng.dma_start(
            odst(b)[:, t0 : t0 + 4, :], wsrc(b)
        ).then_inc(sem, 16)

    with nc.Block() as block:

        @block.sync
        def _(sync):
            load(sync, 0, 0)
            load(sync, 0, 2)
            load(sync, 1, 1)
            load(sync, 1, 3)
            sync.wait_ge(s_wide, 1)
            store(sync, 0, 0, s_sp)
            store(sync, 0, 8, s_sp)
            sync.wait_ge(s_wide, 2)
            store(sync, 1, 0, s_sp)
            store(sync, 1, 8, s_sp)
            sync.wait_ge(s_sp, 64)

        @block.scalar
        def _(scalar):
            load(scalar, 0, 1)
            load(scalar, 0, 3)
            load(scalar, 1, 0)
            load(scalar, 1, 2)
            for b in range(BPC):
                # widen: replicate psum mean tile 4x into wide[b]
                scalar.wait_ge(s_mm, b + 1)
                scalar.copy(wide[b][:, 0:P], pss[b][:])
                scalar.copy(wide[b][:, P : 2 * P], pss[b][:])
                scalar.copy(wide[b][:, 2 * P : 3 * P], pss[b][:])
                scalar.copy(wide[b][:, 3 * P : 4 * P], pss[b][:]).then_inc(
                    s_wide, 1
                )
                scalar.wait_ge(s_wide, b + 1)
                store(scalar, b, 4, s_act)
                store(scalar, b, 12, s_act)
            scalar.wait_ge(s_act, 64)

        @block.vector
        def _(vector):
            vector.memset(w[:], 1.0 / SK).then_inc(s_w, 1)

            def a1(b, c):
                # level-1: (128, 512) chunk -> 256 partial sums
                vector.wait_ge(s_ld[b][c], 16)
                lo = 512 * c
                vector.tensor_add(
                    lv1[b][:, 256 * c : 256 * c + 256],
                    xts[b][:, lo : lo + 256],
                    xts[b][:, lo + 256 : lo + 512],
                )

            def a2(b, c):
                # level-2: 256 -> 128 (chunk sum s_c)
                vector.tensor_add(
                    qac[b][:, 128 * c : 128 * c + 128],
                    lv1[b][:, 256 * c : 256 * c + 128],
                    lv1[b][:, 256 * c + 128 : 256 * c + 256],
                )

            def dummy():
                # spacer: keeps dependent ops >= 2 apart so no pipe drain
                vector.tensor_copy(dum[:, 0:8], qac[0][:, 0:8])

            def batch(b):
                # interleaved: every dependent pair has >= 2 ops between
                a1(b, 0)                                       # L1 c0
                a1(b, 1)                                       # L1 c1
                a2(b, 0)                                       # s0
                a1(b, 2)                                       # L1 c2
                a2(b, 1)                                       # s1
                a2(b, 2)                                       # s2
                vector.tensor_add(                             # F1 = s0+s1
                    pr[b][:, 0:128], qac[b][:, 0:128], qac[b][:, 128:256]
                )
                dummy()
                vector.tensor_add(                             # F2 = F1+s2
                    pr[b][:, 128:256], pr[b][:, 0:128], qac[b][:, 256:384]
                )
                a1(b, 3)                                       # L1 c3
                dummy()
                a2(b, 3)                                       # s3
                dummy()
                vector.tensor_add(                             # acc = F2+s3
                    acc[b][:], pr[b][:, 128:256], qac[b][:, 384:512]
                ).then_inc(s_dve, 1)

            batch(0)
            batch(1)

        @block.tensor
        def _(tensor):
            tensor.wait_ge(s_w, 1)
            for b in range(BPC):
                tensor.wait_ge(s_dve, b + 1)
                nc.tensor.matmul(
                    pss[b][:], w[:], acc[b][:], start=True, stop=True
                ).then_inc(s_mm, 1)

    nc.compile()
    return nc


def kernel(query=None, key=None, value=None, q_param=None, _trace=False):
    from concourse.bass_utils import run_bass_kernel_spmd

    global LAST_RESULT

    value = np.ascontiguousarray(np.asarray(value, dtype=np.float32))
    assert value.shape == (B, SK, DV), value.shape

    nc = _build_nc()
    shards = value.reshape(N_CORES, BPC, SK, DV)
    in_maps = [{"value": shards[i]} for i in range(N_CORES)]

    LAST_RESULT = run_bass_kernel_spmd(
        nc, in_maps, list(range(N_CORES)), trace=_trace
    )
    return np.concatenate(
        [LAST_RESULT.results[i]["out"] for i in range(N_CORES)], axis=0
    )



# revision 3
# speedup vs baseline: 1.0592x; 1.0592x over previous
"""Trainium2 Bass kernel for nn_Attention_39676907884025.

out[b, q, :] = (1/SK) * sum_k value[b, k, :] for every q: q_param (1x1) is
broadcast over query and key, the score matrix is constant along the softmax
axis, and softmax of a constant row is exactly uniform. Only `value` touches
the device; batch B=16 is data-parallel over 8 cores (2 per core).

Raw bacc, hand-scheduled. v2 pipeline (per core, 2 batches):
  - 8 load chunks (256 KB, 2 KB descriptors) on the two HWDGE rings
    (SP 4 + ACT 4), batch-0 chunks first so its store phase overlaps
    batch-1 loads. Dedicated completion sem per chunk.
  - DVE pairwise-adds each chunk (128,512)f32 -> (128,256)bf16 as it
    lands (all ops independent -> no same-engine pipeline drains).
  - PE accumulates the 8 bf16 blocks per batch into one fp32 PSUM tile
    with a constant 1/SK stationary: partition-reduce + chunk-fold +
    broadcast of the mean to all 128 rows in one accumulation group.
  - ACT is the only engine allowed to wait on the PE semaphore (other
    engines waiting on PE event-accel sems hang the device); it relays
    via sem_inc to Pool, which widens PSUM -> (128,512) bf16.
  - Stores in bf16 (host upcasts; |mean| error ~0.3% << 2e-2 budget):
    4 x 128 KB per batch (1 KB descriptors) on SP+ACT.
"""

import sys

import numpy as np

if "/opt/trn_rl_repo" not in sys.path:
    sys.path.insert(0, "/opt/trn_rl_repo")

B, SQ, SK, D, DV = 16, 2048, 2048, 128, 128
N_CORES = 8
BPC = B // N_CORES  # batches per core
P = 128

LAST_RESULT = None  # BassKernelResults of the most recent run (for profiling)


def _build_nc():
    import concourse.bacc as bacc
    import concourse.mybir as mybir

    f32 = mybir.dt.float32
    bf16 = mybir.dt.bfloat16
    nc = bacc.Bacc("TRN2", target_bir_lowering=False)

    val = nc.dram_tensor("value", [BPC, SK, DV], f32, kind="ExternalInput")
    out = nc.dram_tensor("out", [BPC, SQ, DV], bf16, kind="ExternalOutput")

    w = nc.alloc_sbuf_tensor("w_const", [P, P], bf16)
    xts = [nc.alloc_sbuf_tensor(f"xt{b}", [P, SK], f32) for b in range(BPC)]
    # pairwise sums per chunk c: bf16 at [256c, 256c+256)
    lv1 = [nc.alloc_sbuf_tensor(f"lv1_{b}", [P, 1024], bf16) for b in range(BPC)]
    wide = [nc.alloc_sbuf_tensor(f"wide{b}", [P, 512], bf16) for b in range(BPC)]
    pss = [nc.alloc_psum_tensor(f"ps{b}", [P, P], f32) for b in range(BPC)]

    s_ld = [
        [nc.alloc_semaphore(f"s_ld_{b}_{c}") for c in range(4)] for b in range(BPC)
    ]
    s_w = nc.alloc_semaphore("s_w")
    s_dve = [nc.alloc_semaphore(f"s_dve_{b}") for b in range(BPC)]
    s_mm = nc.alloc_semaphore("s_mm")
    s_rel = nc.alloc_semaphore("s_rel")
    s_wide = [nc.alloc_semaphore(f"s_wide_{b}") for b in range(BPC)]
    s_st_sp = nc.alloc_semaphore("s_st_sp")
    s_st_act = nc.alloc_semaphore("s_st_act")

    def xdst(b):
        return xts[b][:].rearrange("p (t d) -> p t d", d=DV)

    def xsrc(b):
        return val[b].rearrange("(p t) d -> p t d", p=P)

    def odst(b):
        return out[b].rearrange("(p t) d -> p t d", p=P)

    def wsrc(b):
        return wide[b][:].rearrange("p (t d) -> p t d", d=DV)

    def load(eng, b, c):
        return eng.dma_start(
            xdst(b)[:, 4 * c : 4 * c + 4, :], xsrc(b)[:, 4 * c : 4 * c + 4, :]
        ).then_inc(s_ld[b][c], 16)

    def store(eng, b, t0, sem):
        return eng.dma_start(
            odst(b)[:, t0 : t0 + 4, :], wsrc(b)
        ).then_inc(sem, 16)

    # chunk arrival order: SP carries c0,c1 and ACT c2,c3, the queues drain
    # round-robin, so chunks land in (c0,c2) then (c1,c3) pairs
    CORD = [0, 2, 1, 3]

    with nc.Block() as block:

        @block.sync
        def _(sync):
            load(sync, 0, 0)
            load(sync, 0, 1)
            load(sync, 1, 0)
            load(sync, 1, 1)
            for b in range(BPC):
                sync.wait_ge(s_wide[b], 1)
                store(sync, b, 0, s_st_sp)
                store(sync, b, 8, s_st_sp)
            sync.wait_ge(s_st_sp, 64)

        @block.scalar
        def _(scalar):
            load(scalar, 0, 2)
            load(scalar, 0, 3)
            load(scalar, 1, 2)
            load(scalar, 1, 3)
            for b in range(BPC):
                # sole engine waiting on the PE semaphore; relay to Pool
                scalar.wait_ge(s_mm, b + 1)
                scalar.sem_inc(s_rel, 1)
                scalar.wait_ge(s_wide[b], 1)
                store(scalar, b, 4, s_st_act)
                store(scalar, b, 12, s_st_act)
            scalar.wait_ge(s_st_act, 64)

        @block.vector
        def _(vector):
            def l1(b, c):
                lo = 512 * c
                vector.wait_ge(s_ld[b][c], 16)
                vector.tensor_add(
                    lv1[b][:, 256 * c : 256 * c + 256],
                    xts[b][:, lo : lo + 256],
                    xts[b][:, lo + 256 : lo + 512],
                ).then_inc(s_dve[b], 1)

            def widen(b):
                # GPSIMD can't touch PSUM; DVE is the widener. It may only
                # wait on the ACT relay sem, never on the PE sem directly.
                vector.wait_ge(s_rel, b + 1)
                vector.tensor_copy(wide[b][:, 0:P], pss[b][:])
                vector.tensor_copy(wide[b][:, P : 2 * P], pss[b][:])
                vector.tensor_copy(wide[b][:, 2 * P : 3 * P], pss[b][:])
                vector.tensor_copy(wide[b][:, 3 * P : 4 * P], pss[b][:]).then_inc(
                    s_wide[b], 1
                )

            vector.memset(w[:], 1.0 / SK).then_inc(s_w, 1)
            for c in CORD:
                l1(0, c)
            l1(1, 0)
            widen(0)
            l1(1, 2)
            l1(1, 1)
            l1(1, 3)
            widen(1)

        @block.tensor
        def _(tensor):
            tensor.wait_ge(s_w, 1)
            for b in range(BPC):
                nmm = 0
                for i, c in enumerate(CORD):
                    tensor.wait_ge(s_dve[b], i + 1)
                    for k in (2 * c, 2 * c + 1):
                        nmm += 1
                        mm = nc.tensor.matmul(
                            pss[b][:],
                            w[:],
                            lv1[b][:, 128 * k : 128 * k + 128],
                            start=(nmm == 1),
                            stop=(nmm == 8),
                        )
                        if nmm == 8:
                            mm.then_inc(s_mm, 1)

    nc.compile()
    return nc


def kernel(query=None, key=None, value=None, q_param=None, _trace=False):
    from concourse.bass_utils import run_bass_kernel_spmd

    global LAST_RESULT

    value = np.ascontiguousarray(np.asarray(value, dtype=np.float32))
    assert value.shape == (B, SK, DV), value.shape

    nc = _build_nc()
    shards = value.reshape(N_CORES, BPC, SK, DV)
    in_maps = [{"value": shards[i]} for i in range(N_CORES)]

    LAST_RESULT = run_bass_kernel_spmd(
        nc, in_maps, list(range(N_CORES)), trace=_trace
    )
    return np.concatenate(
        [
            np.asarray(LAST_RESULT.results[i]["out"]).astype(np.float32)
            for i in range(N_CORES)
        ],
        axis=0,
    )


# revision 6
# speedup vs baseline: 1.1369x; 1.0733x over previous
"""Trainium2 Bass kernel for nn_Attention_39676907884025.

out[b, q, :] = (1/SK) * sum_k value[b, k, :] for every q: q_param (1x1) is
broadcast over query and key, the score matrix is constant along the softmax
axis, and softmax of a constant row is exactly uniform. Only `value` touches
the device; batch B=16 is data-parallel over 8 cores (2 per core).

Raw bacc, hand-scheduled, NO nc.Block. Rationale (from perfetto traces of
the previous versions):
  - The NEFF epilogue makes every engine serially reset its fixed bank of
    ~50 semaphores (0.05-0.13 us each => 2.2-6.5 us per engine). With
    nc.Block, its exit barrier forces all engines to finish the body first,
    so the slowest reset chain lands entirely after the last store
    (~8 us of pure postamble). Emitting raw per-engine streams (no block,
    no exit barrier) lets each engine start its resets right after its own
    last instruction, overlapping them with the DMA tail. The NEFF's own
    final all-engine barrier before the loop-back jump still serializes
    executions, and the entry barrier isolates re-runs.
  - This requires bank-aware semaphore placement: an engine resets its bank
    whenever IT finishes, so a semaphore may only live in bank X if its
    last wait/increment is causally ordered before engine X's last body
    instruction. Banks: PE S[7:54], ACT S[54:105], Pool S[105:156],
    DVE S[156:207], SP S[207:256]; the user pool starts at 155 (Pool bank
    tail - burn it, Pool's body is empty and it resets almost at t=0).
  - All DMA on one HWDGE queue (SP): dma_start issue cost (~0.6 us) pays a
    SHARED HWDGE unit, so spreading across engines doesn't parallelize it,
    and a single queue still fans out over all 16 SDMA engines at full
    HBM rate while making chunk completion strictly FIFO (better
    pipelining than 2 queues' round-robin). 4 load chunks per batch
    (256 KB, 2 KB descriptors) for reduce overlap; ONE store per batch
    via a stride-0 broadcast source AP (1024 x 512 B descriptors) so the
    mean tile only needs 2 replicas in SBUF.
  - DVE pairwise-adds each chunk (128,512)f32 -> (128,256)bf16 as it
    lands; PE accumulates the 4 bf16 blocks per batch into a (128,256)
    fp32 PSUM tile with a constant 1/SK stationary (partition-reduce +
    broadcast); DVE folds psum halves -> (128,256) bf16 wide tile (two
    independent adds). ACT's only job is relaying the PE semaphore to DVE
    (only ACT may wait on PE sems - other engines hang the device).
  - Stores in bf16 (host upcasts; mean error ~0.3% << 2e-2 budget).
"""

import sys

import numpy as np

if "/opt/trn_rl_repo" not in sys.path:
    sys.path.insert(0, "/opt/trn_rl_repo")

B, SQ, SK, D, DV = 16, 2048, 2048, 128, 128
N_CORES = 8
BPC = B // N_CORES  # batches per core
P = 128

LAST_RESULT = None  # BassKernelResults of the most recent run (for profiling)


def _build_nc():
    import concourse.bacc as bacc
    import concourse.mybir as mybir

    f32 = mybir.dt.float32
    bf16 = mybir.dt.bfloat16
    nc = bacc.Bacc("TRN2", target_bir_lowering=False)

    val = nc.dram_tensor("value", [BPC, SK, DV], f32, kind="ExternalInput")
    out = nc.dram_tensor("out", [BPC, SQ, DV], bf16, kind="ExternalOutput")

    w = nc.alloc_sbuf_tensor("w_const", [P, P], bf16)
    xts = [nc.alloc_sbuf_tensor(f"xt{b}", [P, SK], f32) for b in range(BPC)]
    # pairwise sums per chunk c: bf16 at [256c, 256c+256)
    lv1 = [nc.alloc_sbuf_tensor(f"lv1_{b}", [P, 1024], bf16) for b in range(BPC)]
    # two replicas of the folded bf16 mean row
    wide = [nc.alloc_sbuf_tensor(f"wide{b}", [P, 256], bf16) for b in range(BPC)]
    pss = [nc.alloc_psum_tensor(f"ps{b}", [P, P], f32) for b in range(BPC)]

    # --- bank-aware semaphore allocation (pool pops 155, 156, ... in order)
    def sem(name, expect):
        s = nc.alloc_semaphore(name)
        assert s.num == expect, (name, s.num, expect)
        return s

    sem("dummy_pool_bank", 155)  # Pool resets S[155] at ~t=0; never use it
    # DVE bank S[156:207]: last waits/incs all causally precede DVE's last
    # fold (s_ld/s_rel: DVE's own waits; s_w/s_dve: PE consumes before its
    # stop-matmul -> s_mm -> relay -> DVE fold; s_mm: ACT consumes before
    # relay -> DVE fold).
    s_ld = [[sem(f"s_ld_{b}_{c}", 156 + 4 * b + c) for c in range(4)] for b in range(BPC)]
    s_w = sem("s_w", 164)
    s_dve = [sem(f"s_dve_{b}", 165 + b) for b in range(BPC)]
    s_mm = sem("s_mm", 167)
    s_rel = sem("s_rel", 168)
    for i in range(169, 207):  # burn the rest of the DVE bank
        sem(f"dummy_{i}", i)
    # SP bank S[207:256]: SP's own final waits consume these
    s_wide = [sem(f"s_wide_{b}", 207 + b) for b in range(BPC)]
    s_st = sem("s_st", 209)

    def xdst(b):
        return xts[b][:].rearrange("p (t d) -> p t d", d=DV)

    def xsrc(b):
        return val[b].rearrange("(p t) d -> p t d", p=P)

    # --- SP: all loads, both stores, final completion wait
    for b in range(BPC):
        for c in range(4):
            nc.sync.dma_start(
                xdst(b)[:, 4 * c : 4 * c + 4, :], xsrc(b)[:, 4 * c : 4 * c + 4, :]
            ).then_inc(s_ld[b][c], 16)
    for b in range(BPC):
        nc.sync.wait_ge(s_wide[b], 1)
        nc.sync.dma_start(
            out[b].rearrange("(p t u) d -> p t (u d)", p=P, t=8),
            wide[b][:][:, None, :].to_broadcast((P, 8, 256)),
        ).then_inc(s_st, 16)
    nc.sync.wait_ge(s_st, 32)

    # --- ACT: sole waiter on the PE semaphore; relays to DVE
    for b in range(BPC):
        nc.scalar.wait_ge(s_mm, b + 1)
        nc.scalar.sem_inc(s_rel, 1)

    # --- DVE: L1 pairwise adds (f32 -> bf16) + psum fold into wide
    nc.vector.memset(w[:], 1.0 / SK).then_inc(s_w, 1)
    for b in range(BPC):
        for c in range(4):
            lo = 512 * c
            nc.vector.wait_ge(s_ld[b][c], 16)
            nc.vector.tensor_add(
                lv1[b][:, 256 * c : 256 * c + 256],
                xts[b][:, lo : lo + 256],
                xts[b][:, lo + 256 : lo + 512],
            ).then_inc(s_dve[b], 1)
        nc.vector.wait_ge(s_rel, b + 1)
        nc.vector.tensor_copy(wide[b][:, 0:P], pss[b][:])
        nc.vector.tensor_copy(wide[b][:, P : 2 * P], pss[b][:]).then_inc(
            s_wide[b], 1
        )

    # --- PE: accumulate the 8 chunk blocks into the psum mean tile
    nc.tensor.wait_ge(s_w, 1)
    for b in range(BPC):
        for k in range(8):
            if k % 2 == 0:
                nc.tensor.wait_ge(s_dve[b], k // 2 + 1)
            mm = nc.tensor.matmul(
                pss[b][:],
                w[:],
                lv1[b][:, 128 * k : 128 * k + 128],
                start=(k == 0),
                stop=(k == 7),
            )
            if k == 7:
                mm.then_inc(s_mm, 1)

    nc.compile()
    return nc


def kernel(query=None, key=None, value=None, q_param=None, _trace=False):
    from concourse.bass_utils import run_bass_kernel_spmd

    global LAST_RESULT

    value = np.ascontiguousarray(np.asarray(value, dtype=np.float32))
    assert value.shape == (B, SK, DV), value.shape

    nc = _build_nc()
    shards = value.reshape(N_CORES, BPC, SK, DV)
    in_maps = [{"value": shards[i]} for i in range(N_CORES)]

    LAST_RESULT = run_bass_kernel_spmd(
        nc, in_maps, list(range(N_CORES)), trace=_trace
    )
    return np.concatenate(
        [
            np.asarray(LAST_RESULT.results[i]["out"]).astype(np.float32)
            for i in range(N_CORES)
        ],
        axis=0,
    )
